# revision 1
# baseline (speedup 1.0000x reference)
"""Trainium2 Bass kernel for MultiHeadCrossLayerHoughNetSpatialRelation.

Computation (per batch b):
  votes = queries @ vote_w.T            [N, 2V]
  vote_pos = ref[:, :2] + votes         -> N*V (n,v) points
  d2[(n,v), j] = ||vote_pos - center_j||^2
  scores = exp(-d2 / (2 sigma_{i mod N}^2))
  imap[n, j] = sum_v scores;  imap /= rowsum;  x = 1 - imap
  out[h, n, j] = relu( sum_f proj_w[h,f] * sine_feat_f(x) + proj_b[h] )

The 16 sine features use freqs 100/10^(k/2), k=0..7. The four high-freq
pairs (f >= 3.16) are computed exactly; the four low-freq pairs (f <= 1)
are least-squares-folded onto the basis {1, x, kept sines} (sup error
<= ~1.6e-3 before weighting). This gives 9 on-chip features per (n,j):
8 hard sines + one linear carrier (sin(eps*x)/eps).

Sharding: 8 cores = 4 batches x 2 n-halves; each core owns 450 n-rows
(padded to 462 = 33 tiles of 14 n) x all 900 j columns. Votes come in
57 blocks of 128 partitions (16 votes x 8 n).

Device pipeline (software-pipelined flat schedule, "half" units):
  - ramp: inputs land in dependency order (tR/tSc + a small 2-block L
    chunk first, later chunks behind the tile constants, all split
    across the SP/ACT DMA queues); a 1-column dummy Exp absorbs the
    1.3us activation-table load while the first d2 inputs are in flight
  - per block: d2 via K=18 fp16 hi/mid/lo split matmul (fp32-accurate,
    full PE rate, separate L-chunk tiles so the first matmul only waits
    its own chunk), ACT Exp with per-partition scale (-1/2sigma^2) and
    accum_out row-sums (scores stored float32r); batched GSc matmul
    folds the 16 vote rowsums per n, DVE max+recip gives 1/rowsum,
    Pool broadcasts it into the per-tile G2 weights
  - per tile, split into two independent 1-bank PSUM units (columns
    0:512 / 512:900, 4 units rotating through 4 banks; the d2 blocks
    own the other 4): float32r group matmuls fuse vote-sum,
    normalization and per-feature frequency scaling into sine arguments
    in turns; a K=2 matmul adds hi/lo-split phase biases; DVE
    magic-number round -> r, a -identity matmul accumulates -r (args
    land in [-0.5, 0.5] turns, the HW sin2pi table's exact domain);
    one ACT Sin2pi evaluates all 9 features (cos via +0.25 turns) into
    fp16; the fp16 block-diagonal projection matmul (9 features -> 8
    heads) reuses the unit's own PSUM bank; DVE dual-op fuses
    per-partition bias + relu, writing fp16
  - fp16 output DMA to HBM per half-unit (host upcasts to fp32)
  Engine budget per core: ACT ~105us busy (57 Exp + 66 half-Sin, one
  shared exp_and_friends table), PE ~89us, DVE ~85us, Pool ~9us,
  SP/DMA ~42us. Block emission runs one extra block ahead of the
  2-tile lookahead (la_extra=1) -- that single block of slack covers
  the zero-margin Exp->GSc->prep->group-mm->round->Sin chain and
  removes its periodic 2.3us ACT bubble; a 6-matmul PE warm-up burst
  starts the p-state ramp during the input DMAs. CoreSim cost model:
  ~131us per invocation (~123us/rep steady state) vs ~206us for the
  session-start baseline.
"""

import sys

sys.path.insert(0, "/opt/trn_rl_repo")

import numpy as np
import concourse.bass as bass
import concourse.tile as tile
import concourse.mybir as mybir
import concourse.bass2jax as bass2jax
from concourse.bass_utils import run_bass_kernel_spmd

F32 = mybir.dt.float32
F32R = mybir.dt.float32r
F16 = mybir.dt.float16
AF = mybir.ActivationFunctionType
ALU = mybir.AluOpType

N_CORES = 8
B, N, D, V, H = 4, 900, 256, 16, 8
M = 900            # j columns
NH = 450           # real n rows per core
NPT = 14           # n rows per feature tile
NF = 9             # on-chip features per n
TILES = 33
NC_PAD = NPT * TILES          # 462
BLOCKS = 57                   # ceil(450/8) -> 57 blocks of 8 n (456 padded);
                              # tile rows 456-461 get no block (pad, discarded)
NB_PAD = 8 * BLOCKS           # 456
MAGIC = float(1.5 * 2**23)
EPS_LIN = 0.05
TWO_PI = 2.0 * np.pi
CH = [(0, 512), (512, M)]

_k8 = np.arange(8)
FREQS = 100.0 / (10000.0 ** (_k8 / 8.0))     # 100, 31.6, 10, 3.16, 1, ...
# feature slots: [sin0, cos0, sin1, cos1, sin2, cos2, sin3, cos3, lin]
SLOT_FREQ = np.array([FREQS[0], FREQS[0], FREQS[1], FREQS[1], FREQS[2],
                      FREQS[2], FREQS[3], FREQS[3], EPS_LIN])
SLOT_PHASE = np.array([0.0, np.pi / 2, 0.0, np.pi / 2, 0.0, np.pi / 2,
                       0.0, np.pi / 2, 0.0])

# ---------------------------------------------------------------------------
# walrus workarounds
# ---------------------------------------------------------------------------
_wsplit_uid = [0]


def _split_sync_waits(nc):
    for fn in nc.m.functions:
        for blk in fn.blocks:
            new_list = []
            changed = False
            for inst in blk.instructions:
                si = inst.sync_info
                waits = list(si.on_wait) if (si and si.on_wait) else []
                if len(waits) > 1:
                    changed = True
                    for w in waits[:-1]:
                        _wsplit_uid[0] += 1
                        nop = mybir.InstNoOp(
                            name=f"wsplit-{_wsplit_uid[0]}", ins=[], outs=[]
                        )
                        nop.engine = inst.engine
                        nop.sync_info = mybir.SyncInfo(on_wait=[w], on_update=[])
                        new_list.append(nop)
                    inst.sync_info = mybir.SyncInfo(
                        on_wait=[waits[-1]], on_update=list(si.on_update or [])
                    )
                new_list.append(inst)
            if changed:
                blk.instructions = new_list


_orig_compile = bass2jax.compile_bir_kernel
_patch_installed = [False]


def _install_compile_patch():
    if _patch_installed[0]:
        return

    def _patched(bir_json, *a, **k):
        bir_json = bir_json.replace(b'"Arctan"', b'"Sin2pi"')
        return _orig_compile(bir_json, *a, **k)

    bass2jax.compile_bir_kernel = _patched
    _patch_installed[0] = True


def _contributions():
    """(tau, blk, delta) for every 8-n block overlapping a 14-n tile.

    Blocks past BLOCKS-1 cover only pad rows (n >= 456 > NH=450); tiles
    lose those contributions and the pad rows decode to garbage that the
    host slices away."""
    out = []
    for tau in range(TILES):
        lo, hi = NPT * tau, NPT * tau + NPT
        b0 = lo // 8
        b1 = min((hi - 1) // 8, BLOCKS - 1)
        for blk in range(b0, b1 + 1):
            out.append((tau, blk, 8 * blk - lo))
    return out


CONTRIBS = _contributions()
DELTAS = sorted({d for _, _, d in CONTRIBS})

# ---------------------------------------------------------------------------
# device program
# ---------------------------------------------------------------------------
_prog_cache = {}

# bench knobs (bench scripts may override before building)
VARIANT = {"sin_func": "sin2pi", "accum": False, "gpsimd": False, "dma_out": True,
           "exp": True, "g2": True, "w2": True, "rround": True, "group": 7, "stage": 6,
           "bench": False, "rr_eng": "dve", "out_eng": "dve", "act_bias": False,
           "sched": "flat", "red_eng": "act", "g2_eng": "pool",
           "rr_mode": "negi", "half": True, "prep_div": False,
           "out_f16": True, "dma_half": True, "la_extra": 1, "pe_warm": 6}

CONTRIBS_BY_TILE = {}
for _t, _b, _d in _contributions():
    CONTRIBS_BY_TILE.setdefault(_t, []).append((_b, _d))
LAST_BLK_OF_TILE = {t: max(b for b, _ in v) for t, v in CONTRIBS_BY_TILE.items()}


def _build_program(repeat=1):
    key = ("nc", repeat, tuple(sorted(VARIANT.items())))
    if key in _prog_cache:
        return _prog_cache[key]
    nc = bass.Bass("TRN2", target_bir_lowering=False, debug=False,
                   num_devices=N_CORES)

    L_in = nc.dram_tensor("L", [18, BLOCKS * 128], F16, kind="ExternalInput")
    R_in = nc.dram_tensor("R", [18, M], F16, kind="ExternalInput")
    Sc_in = nc.dram_tensor("Sc", [128, BLOCKS], F32, kind="ExternalInput")
    G2c_in = nc.dram_tensor("G2c", [128, len(DELTAS) * 128], F32,
                            kind="ExternalInput")
    GSc_in = nc.dram_tensor("GSc", [128, 128], F32, kind="ExternalInput")
    W2c_in = nc.dram_tensor("W2c", [128, 128], F16, kind="ExternalInput")
    bp_in = nc.dram_tensor("bp", [2, 128], F32, kind="ExternalInput")
    bpc_in = nc.dram_tensor("bpc", [128, 3], F32, kind="ExternalInput")
    bo_in = nc.dram_tensor("bo", [128, 1], F32, kind="ExternalInput")
    negI_in = nc.dram_tensor("negI", [128, 128], F32, kind="ExternalInput")

    bench = VARIANT.get("bench", False)
    OF16 = VARIANT.get("out_f16", False)
    out_d = nc.dram_tensor("out", [NC_PAD, H, M], F16 if OF16 else F32,
                           kind="Internal" if bench else "ExternalOutput")
    dum_d = (nc.dram_tensor("dum", [1, 8], F32, kind="ExternalOutput")
             if bench else None)

    didx = {d: i for i, d in enumerate(DELTAS)}

    with tile.TileContext(nc) as tc:
        with (
            tc.tile_pool(name="cst", bufs=1) as cst,
            tc.tile_pool(name="scp", bufs=16) as scp,
            tc.tile_pool(name="smp", bufs=16) as smp,
            tc.tile_pool(name="g2p", bufs=3) as g2p,
            tc.tile_pool(name="sb", bufs=2) as sb,
            tc.tile_pool(name="d2ps", bufs=2, space="PSUM") as d2ps,
            tc.tile_pool(name="tps", bufs=1, space="PSUM") as tps,
            tc.tile_pool(name="ops", bufs=1, space="PSUM") as ops,
        ):
            # load order matters: the first d2 matmul needs tR + the first
            # chunk of tL; Exp needs tSc. Everything else arrives later.
            # Keep the ACT queue clear so the first Exp (plus its act-table
            # load) can issue as soon as the first d2 block lands.
            # tR split per matmul chunk; first L chunk tiny (2 blocks) so the
            # first d2 matmul's inputs land as early as possible.
            tR_t = [cst.tile([18, c1 - c0], F16, tag=f"tR{i}",
                             name=f"tRc{i}")
                    for i, (c0, c1) in enumerate(CH)]
            nc.sync.dma_start(tR_t[0][:], R_in[:, CH[0][0]:CH[0][1]])

            def tR_ap(ci, w):
                return tR_t[ci][:, 0:w]

            _lsz = [2] + [8] * ((BLOCKS - 2 + 7) // 8)
            while sum(_lsz) > BLOCKS:
                _lsz[-1] -= 1
            _lof = [0]
            for s in _lsz:
                _lof.append(_lof[-1] + s)
            tL_t = [cst.tile([18, s * 128], F16, tag=f"tL{c}",
                             name=f"tLc{c}") for c, s in enumerate(_lsz)]
            nc.scalar.dma_start(
                tL_t[0][:], L_in[:, 0 : _lsz[0] * 128]
            )
            nc.sync.dma_start(tR_t[1][:], R_in[:, CH[1][0]:CH[1][1]])
            tSc = cst.tile([128, BLOCKS], F32)
            nc.sync.dma_start(tSc[:], Sc_in[:])
            # absorb the 1.3us activation-table load off the critical ramp:
            # a 1-column Exp issued while the first d2 inputs are in flight
            wsrc = cst.tile([128, 1], F32)
            nc.vector.memset(wsrc[:], 0.0)
            wdst = cst.tile([128, 1], F16)
            nc.scalar.activation(wdst[:], wsrc[:], AF.Exp)
            def _lchunk(c, eng):
                lc = _lof[c] * 128
                hi = (_lof[c] + _lsz[c]) * 128
                eng.dma_start(tL_t[c][:], L_in[:, lc:hi])

            _lchunk(1, nc.sync)
            tGSc = cst.tile([128, 128], F32)
            nc.sync.dma_start(tGSc[:], GSc_in[:])
            tG2c = cst.tile([128, len(DELTAS) * 128], F32)
            nc.sync.dma_start(tG2c[:], G2c_in[:])
            _lchunk(2, nc.sync)

            _blk2c = []
            for c, s in enumerate(_lsz):
                _blk2c += [(c, i) for i in range(s)]

            def tL_ap(b):
                c, o = _blk2c[b]
                return tL_t[c][:, o * 128 : (o + 1) * 128]
            tW2c = cst.tile([128, 128], F16)
            nc.sync.dma_start(tW2c[:], W2c_in[:])
            tbo = cst.tile([128, 1], F32)
            nc.sync.dma_start(tbo[:], bo_in[:])

            tmpI = cst.tile([128, 128], F32)
            nc.sync.dma_start(tmpI[:], negI_in[:])
            tnegI = cst.tile([128, 128], F32R)
            nc.vector.tensor_copy(tnegI[:], tmpI[:])

            # late L chunks ride behind the tile-pipeline constants, all on
            # SP so the ACT queue stays clear for the first real Exps
            for _c in range(3, len(_lsz)):
                _lchunk(_c, nc.sync)

            ACT_BIAS = VARIANT["act_bias"]
            if ACT_BIAS:
                # bpc[:,0] = MAGIC + bp (per-feature-partition round bias),
                # bpc[:,1] = bp in turns, bpc[:,2] = 2*pi*bp
                tbpc = cst.tile([128, 3], F32)
                nc.sync.dma_start(tbpc[:], bpc_in[:])
            else:
                tbp = cst.tile([2, 128], F32)
                nc.sync.dma_start(tbp[:], bp_in[:])
                tbh = cst.tile([1, 128], F32R)
                nc.vector.tensor_copy(tbh[:], tbp[0:1, :])
                tbh32 = cst.tile([1, 128], F32)
                nc.vector.tensor_copy(tbh32[:], tbh[:])
                tbl32 = cst.tile([1, 128], F32)
                nc.vector.scalar_tensor_tensor(
                    tbl32[:], tbp[0:1, :], 1.0, tbh32[:], ALU.mult, ALU.subtract
                )
                tbl = cst.tile([1, 128], F32R)
                nc.vector.tensor_copy(tbl[:], tbl32[:])
                tbp2 = cst.tile([2, 128], F32R)
                nc.sync.dma_start(tbp2[0:1, :], tbh[:])
                nc.sync.dma_start(tbp2[1:2, :], tbl[:])

                tones32 = cst.tile([2, M], F32)
                nc.vector.memset(tones32[:], 1.0)
                tones = cst.tile([2, M], F32R)
                nc.vector.tensor_copy(tones[:], tones32[:])

            # Preallocated rotating buffers (pool.tile() alloc/release has
            # ~0.2 ms overhead per tile on this runtime, so allocate once
            # and cycle manually; Tile's access tracking provides WAR deps).
            ND2, NSC, NG2, NRR = 2, 16, 6, 3
            d2p_t = [d2ps.tile([128, 1024], F32, tag="d2", name=f"d2b{i}",
                               bufs=ND2)
                     for i in range(ND2)]
            sct_t = [scp.tile([128, M],
                              F16 if VARIANT.get("sc_f16", False) else F32R,
                              tag="scores", name=f"scb{i}", bufs=NSC)
                     for i in range(NSC)]
            rsacc_t = [smp.tile([128, 1], F32, tag="rsacc", name=f"rsb{i}",
                                bufs=NSC)
                       for i in range(NSC)]
            invrs_t = [smp.tile([128, 1], F32, tag="invrs", name=f"ivb{i}",
                                bufs=NSC)
                       for i in range(NSC)]
            g2rt_t = [g2p.tile([128, 128], F32R, tag="g2rt", name=f"g2b{i}",
                               bufs=NG2)
                      for i in range(NG2)]
            _half_mode = (VARIANT["sched"] == "flat"
                          and VARIANT.get("half", False))
            tps_t = ([tps.tile([128, 1024], F32, tag="t", name="tpsb",
                               bufs=1)]
                     if not _half_mode else [None])
            rr_t = [sb.tile([128, M], F32R, tag="rr", name=f"rrb{i}",
                            bufs=NRR)
                    for i in range(NRR)]
            ft_t = [sb.tile([128, M], F16, tag="feats", name=f"ftb{i}",
                            bufs=NRR)
                    for i in range(NRR)]
            op_t = ([ops.tile([128, 1024], F32, tag="outp", name="opb",
                              bufs=1)]
                    if not _half_mode else [None])
            ob_t = [sb.tile([128, M], F16 if VARIANT.get("out_f16", False)
                            else F32, tag="outsb", name=f"obb{i}",
                            bufs=NRR)
                    for i in range(NRR)]
            g2ctr = [0]

            # per-block state carried to (up to two) consuming tiles
            blk_scores = {}
            blk_invrs = {}

            def process_block(blk, rep):
                d2p = d2p_t[blk % ND2]
                for ci, (c0, c1) in enumerate(CH):
                    nc.tensor.matmul(
                        d2p[:, c0:c1],
                        tL_ap(blk),
                        tR_ap(ci, c1 - c0),
                        start=True, stop=True,
                    )
                sct = sct_t[blk % NSC]
                rsacc = rsacc_t[blk % NSC]
                if VARIANT["accum"]:
                    nc.scalar.activation(
                        sct[:], d2p[:, 0:M], AF.Exp,
                        scale=tSc[:, blk : blk + 1], accum_out=rsacc[:],
                    )
                else:
                    nc.scalar.activation(
                        sct[:], d2p[:, 0:M], AF.Exp,
                        scale=tSc[:, blk : blk + 1],
                    )
                    nc.vector.tensor_reduce(
                        rsacc[:], sct[:], mybir.AxisListType.X, ALU.add
                    )
                nc.tensor.matmul(
                    d2p[:, 1000:1001], tGSc[:], rsacc[:], start=True, stop=True
                )
                invrs = invrs_t[blk % NSC]
                nc.vector.tensor_scalar(
                    invrs[:], d2p[:, 1000:1001], 1e-12, None, ALU.max
                )
                nc.vector.reciprocal(invrs[:], invrs[:])
                blk_scores[blk] = sct
                blk_invrs[blk] = invrs

            if VARIANT["sched"] == "flat":
                # -----------------------------------------------------------
                # Software-pipelined flat schedule.
                # PSUM: d2 2x[128,1024] + t 2x[128,1024] = 8 banks. The
                # projection matmul reuses the tile's own t-PSUM buffer after
                # the frac op consumed it.
                # Range reduction is elementwise (no PE negI matmul):
                #   rr   = (t + MAGIC) - MAGIC          (round to int)
                #   frac = t - rr                        (into SBUF)
                #   feats = sin2pi(frac)                 (ACT, phases via the
                #                                         K=2 bias matmul)
                # Iteration tau emits: Sin(tau) -> GSc batch (blocks emitted
                # last iter) -> g2rt prep (tau+1) -> d2+Exp+rowsum for tile
                # tau+2's blocks -> group+bias mm (tau+1) -> rr/frac (tau+1)
                # -> proj (tau) -> bias+relu+DMA (tau).
                # -----------------------------------------------------------
                ENG = {"dve": nc.vector, "pool": nc.gpsimd}
                red_e = ENG.get(VARIANT["red_eng"])
                g2_e = ENG[VARIANT["g2_eng"]]
                rr_e = ENG[VARIANT["rr_eng"]]
                out_e = ENG[VARIANT["out_eng"]]
                _sf = VARIANT["sin_func"]

                tps2_t = [tps_t[0], op_t[0]]
                HALF = VARIANT.get("half", False)
                FUSE_SIN = VARIANT.get("fuse_sin", False)
                if HALF and FUSE_SIN:
                    # two 2-bank [128,1024] pair tiles; halves live at column
                    # offsets 0 / 512 of the same memref so per-half ops keep
                    # their subtile independence but Sin reads 0:900 in one
                    # instruction.
                    tqp_t = [
                        tps.tile([128, 1024], F32, tag="tqp0",
                                 name="tqp0", bufs=1),
                        ops.tile([128, 1024], F32, tag="tqp1",
                                 name="tqp1", bufs=1),
                    ]
                    tq_t = [tqp_t[0][:, 0:512], tqp_t[0][:, 512:1024],
                            tqp_t[1][:, 0:512], tqp_t[1][:, 512:1024]]
                elif HALF:
                    # four 1-bank [128,512] units: unit (tau,h) ->
                    # tq_t[(tau%2)*2+h]; tile tau spans halves (0,512),(512,900)
                    tq_t = [
                        tps.tile([128, 512], F32, tag=f"tq{i}",
                                 name=f"tq{i}", bufs=1)
                        for i in range(2)
                    ] + [
                        tps.tile([128, 512], F32, tag=f"tq{i}",
                                 name=f"tq{i}", bufs=1)
                        for i in range(2, 4)
                    ]
                rsg = smp.tile([128, BLOCKS], F32, tag="rsg", name="rsg",
                               bufs=1)
                ivg = smp.tile([128, BLOCKS], F32, tag="ivg", name="ivg",
                               bufs=1)
                fr_t = [sb.tile([128, M], F32, tag="frac", name=f"frb{i}",
                                bufs=NRR)
                        for i in range(NRR)]
                e_emitted = [0]
                g_done = [0]
                g2_of_tile = {}

                def emit_d2exp(upto):
                    for b in range(e_emitted[0], upto):
                        d2p = d2p_t[b % ND2]
                        for ci, (c0, c1) in enumerate(CH):
                            nc.tensor.matmul(
                                d2p[:, c0:c1],
                                tL_ap(b),
                                tR_ap(ci, c1 - c0), start=True, stop=True,
                            )
                        sct = sct_t[b % NSC]
                        red = VARIANT["red_eng"]
                        if red == "mix":
                            red = "act" if b % 2 else "dve"
                        if red == "act":
                            nc.scalar.activation(
                                sct[:], d2p[:, 0:M], AF.Exp,
                                scale=tSc[:, b : b + 1],
                                accum_out=rsg[:, b : b + 1],
                            )
                        else:
                            nc.scalar.activation(
                                sct[:], d2p[:, 0:M], AF.Exp,
                                scale=tSc[:, b : b + 1],
                            )
                            ENG[red].tensor_reduce(
                                rsg[:, b : b + 1], sct[:],
                                mybir.AxisListType.X, ALU.add,
                            )
                    e_emitted[0] = max(e_emitted[0], upto)

                def emit_gsc(upto=None):
                    b0, b1 = g_done[0], (upto if upto is not None
                                         else e_emitted[0])
                    if b1 <= b0:
                        return
                    nb = b1 - b0
                    gout = d2p_t[(b1 - 1) % ND2]
                    nc.tensor.matmul(
                        gout[:, 1000 : 1000 + nb], tGSc[:], rsg[:, b0:b1],
                        start=True, stop=True,
                    )
                    if VARIANT.get("prep_div", True):
                        # rowsum >= ~100*exp(-small) > 0 always; copy to SBUF
                        # and divide inside prep_g2 (drops max+recip hops)
                        nc.vector.tensor_copy(
                            ivg[:, b0:b1], gout[:, 1000 : 1000 + nb]
                        )
                    else:
                        nc.vector.tensor_scalar(
                            ivg[:, b0:b1], gout[:, 1000 : 1000 + nb], 1e-12,
                            None, ALU.max,
                        )
                        nc.vector.reciprocal(ivg[:, b0:b1], ivg[:, b0:b1])
                    g_done[0] = b1

                def prep_g2(tau):
                    lst = []
                    op = (ALU.divide if VARIANT.get("prep_div", True)
                          else ALU.mult)
                    for b, d in CONTRIBS_BY_TILE[tau]:
                        g2rt = g2rt_t[g2ctr[0] % NG2]
                        g2ctr[0] += 1
                        di = didx[d]
                        g2_e.tensor_scalar(
                            g2rt[:], tG2c[:, di * 128 : (di + 1) * 128],
                            ivg[:, b : b + 1], None, op,
                        )
                        lst.append((b, g2rt))
                    g2_of_tile[tau] = lst

                RRM = VARIANT.get("rr_mode", "negi")

                def stage_g(tau):
                    t_ps = tps2_t[tau % 2]
                    conts = g2_of_tile.pop(tau)
                    for c0, c1 in CH:
                        for ci, (b, g2rt) in enumerate(conts):
                            nc.tensor.matmul(
                                t_ps[:, c0:c1], g2rt[:],
                                sct_t[b % NSC][:, c0:c1],
                                start=(ci == 0), stop=False,
                                skip_group_check=True,
                            )
                        nc.tensor.matmul(
                            t_ps[:, c0:c1], tbp2[:], tones[:, c0:c1],
                            start=False, stop=(RRM != "negi"),
                            skip_group_check=True,
                        )

                def stage_rrfrac(tau):
                    t_ps = tps2_t[tau % 2]
                    rr = rr_t[tau % NRR]
                    fr = fr_t[tau % NRR]
                    chunks = CH if VARIANT.get("rr_split", True) else [(0, M)]
                    if RRM == "negi":
                        # round on DVE (may span banks), subtract via PE
                        # accumulate (must stay within one PSUM bank each).
                        for c0, c1 in chunks:
                            rr_e.tensor_scalar(
                                rr[:, c0:c1], t_ps[:, c0:c1], MAGIC, MAGIC,
                                ALU.add, ALU.subtract,
                            )
                        for c0, c1 in CH:
                            nc.tensor.matmul(
                                t_ps[:, c0:c1], tnegI[:], rr[:, c0:c1],
                                start=False, stop=True, skip_group_check=True,
                            )
                        return
                    for c0, c1 in chunks:
                        if RRM == "mod":
                            # t >= 0 always (phases are 0 or pi/2 and the
                            # vote-sum term only subtracts g*x with x<=1), so
                            # frac = t mod 1 lands in [0,1).
                            # (rejected by the walrus DVE ISA check - sim only)
                            rr_e.tensor_scalar(
                                fr[:, c0:c1], t_ps[:, c0:c1], 1.0, None,
                                ALU.mod,
                            )
                            continue
                        rr_e.tensor_scalar(
                            rr[:, c0:c1], t_ps[:, c0:c1], MAGIC, MAGIC,
                            ALU.add, ALU.subtract,
                        )
                        rr_e.scalar_tensor_tensor(
                            fr[:, c0:c1], t_ps[:, c0:c1], 1.0,
                            rr[:, c0:c1], ALU.mult, ALU.subtract,
                        )

                TAIL_CH = CH if VARIANT.get("chunk_tail", False) else [(0, M)]

                def stage_sin(tau):
                    feats = ft_t[tau % NRR]
                    fr = (tps2_t[tau % 2]
                          if VARIANT.get("no_rr", False) or RRM == "negi"
                          else fr_t[tau % NRR])
                    for c0, c1 in TAIL_CH:
                        if _sf == "sin":
                            nc.scalar.activation(
                                feats[:, c0:c1], fr[:, c0:c1], AF.Sin,
                                scale=float(TWO_PI),
                            )
                        else:
                            nc.scalar.activation(
                                feats[:, c0:c1], fr[:, c0:c1], AF.Arctan
                            )

                def _proj_dst(tau):
                    if VARIANT.get("proj_d2", False):   # timing probe only
                        return d2p_t[tau % ND2]
                    return tps2_t[tau % 2]

                def stage_proj(tau):
                    t_ps = _proj_dst(tau)
                    for c0, c1 in CH:
                        nc.tensor.matmul(
                            t_ps[:, c0:c1], tW2c[:],
                            ft_t[tau % NRR][:, c0:c1],
                            start=True, stop=True,
                        )

                def stage_out(tau):
                    out_sb = ob_t[tau % NRR]
                    och = CH if VARIANT.get("out_split", False) else TAIL_CH
                    for c0, c1 in och:
                        out_e.tensor_scalar(
                            out_sb[:, c0:c1], _proj_dst(tau)[:, c0:c1],
                            tbo[:], 0.0, ALU.add, ALU.max,
                        )
                    if VARIANT["dma_out"]:
                        nc.sync.dma_start(
                            bass.AP(out_d, NPT * tau * M * H,
                                    [[M, NPT * H], [1, M]]),
                            out_sb[0 : NPT * H, :],
                        )

                HCH = [(0, 512, 512), (512, 900, 388)]   # (c0, c1, width)

                def _tq(tau, h):
                    return tq_t[(tau % 2) * 2 + h]

                def hstage_g(tau, h):
                    c0, c1, w = HCH[h]
                    tq = _tq(tau, h)
                    conts = (g2_of_tile[tau] if h == 0
                             else g2_of_tile.pop(tau))
                    for ci, (b, g2rt) in enumerate(conts):
                        nc.tensor.matmul(
                            tq[:, 0:w], g2rt[:], sct_t[b % NSC][:, c0:c1],
                            start=(ci == 0), stop=False,
                            skip_group_check=True,
                        )
                    nc.tensor.matmul(
                        tq[:, 0:w], tbp2[:], tones[:, c0:c1],
                        start=False, stop=False, skip_group_check=True,
                    )

                def hstage_rr1(tau, h):
                    c0, c1, w = HCH[h]
                    rr_e.tensor_scalar(
                        rr_t[tau % NRR][:, c0:c1], _tq(tau, h)[:, 0:w],
                        MAGIC, MAGIC, ALU.add, ALU.subtract,
                    )

                def hstage_negi1(tau, h):
                    c0, c1, w = HCH[h]
                    nc.tensor.matmul(
                        _tq(tau, h)[:, 0:w], tnegI[:],
                        rr_t[tau % NRR][:, c0:c1],
                        start=False, stop=True, skip_group_check=True,
                    )

                def hstage_rrnegi(tau, h):
                    c0, c1, w = HCH[h]
                    tq = _tq(tau, h)
                    rr = rr_t[tau % NRR]
                    rr_e.tensor_scalar(
                        rr[:, c0:c1], tq[:, 0:w], MAGIC, MAGIC,
                        ALU.add, ALU.subtract,
                    )
                    nc.tensor.matmul(
                        tq[:, 0:w], tnegI[:], rr[:, c0:c1],
                        start=False, stop=True, skip_group_check=True,
                    )

                def hstage_sin(tau, h):
                    feats = ft_t[tau % NRR]
                    if h is None:        # fused: one pass over both halves
                        src = tqp_t[tau % 2][:, 0:M]
                        dst = feats[:, 0:M]
                    else:
                        c0, c1, w = HCH[h]
                        src = _tq(tau, h)[:, 0:w]
                        dst = feats[:, c0:c1]
                    if _sf == "sin":
                        nc.scalar.activation(
                            dst, src, AF.Sin, scale=float(TWO_PI),
                        )
                    else:
                        nc.scalar.activation(dst, src, AF.Arctan)

                def hstage_proj(tau, h):
                    c0, c1, w = HCH[h]
                    tq = _tq(tau, h)
                    nc.tensor.matmul(
                        tq[:, 0:w], tW2c[:], ft_t[tau % NRR][:, c0:c1],
                        start=True, stop=True,
                    )

                def hstage_out(tau, h):
                    c0, c1, w = HCH[h]
                    tq = _tq(tau, h)
                    out_e.tensor_scalar(
                        ob_t[tau % NRR][:, c0:c1], tq[:, 0:w], tbo[:], 0.0,
                        ALU.add, ALU.max,
                    )

                def hstage_dma(tau, h=None):
                    if not VARIANT["dma_out"]:
                        return
                    # only real n rows leave the core (the host discards
                    # n >= NH anyway); trims the drain-critical last-tile
                    # DMA from 112 to 16 partitions
                    rh = min(NPT, max(0, NH - NPT * tau)) * H
                    if rh <= 0:
                        return
                    if h is None:
                        nc.sync.dma_start(
                            bass.AP(out_d, NPT * tau * M * H,
                                    [[M, rh], [1, M]]),
                            ob_t[tau % NRR][0:rh, :],
                        )
                    else:
                        c0, c1, w = HCH[h]
                        nc.sync.dma_start(
                            bass.AP(out_d, NPT * tau * M * H + c0,
                                    [[M, rh], [1, w]]),
                            ob_t[tau % NRR][0:rh, c0:c1],
                        )

                LA = VARIANT.get("lookahead", 2)
                if HALF:
                    PRO = min(VARIANT.get("pro", 2) - 1, LA - 1)
                    NWARM = VARIANT.get("pe_warm", 0)
                    if NWARM:
                        # dependency-free matmuls keep the PE p-state clock
                        # running while the first input DMAs are in flight,
                        # so the first real d2 matmuls run at full rate
                        wjk = cst.tile([128, 64], F32, name="wjk")
                        nc.vector.memset(wjk[:], 0.0)
                        for _w in range(NWARM):
                            nc.tensor.matmul(
                                tq_t[3][0:64, 0:64], wjk[:, 0:64],
                                wjk[:, 0:64], start=True, stop=True,
                            )
                    for rep in range(repeat):
                        e_emitted[0] = 0
                        g_done[0] = 0
                        emit_d2exp(min(
                            LAST_BLK_OF_TILE[min(PRO, TILES - 1)] + 1
                            + VARIANT.get("pro_extra", 0),
                            BLOCKS,
                        ))
                        emit_gsc()
                        prep_g2(0)
                        for h in (0, 1):
                            hstage_g(0, h)
                            hstage_rrnegi(0, h)
                        for tau in range(TILES):
                            _ord = VARIANT.get("h_ord", "d2_first")
                            if _ord == "spread":
                                hstage_sin(tau, 0)
                                mark = e_emitted[0]
                                if tau + LA < TILES:
                                    emit_d2exp(
                                        LAST_BLK_OF_TILE[tau + LA] + 1
                                    )
                                emit_gsc(mark)
                                if tau + 1 < TILES:
                                    prep_g2(tau + 1)
                                    hstage_g(tau + 1, 0)
                                    hstage_rrnegi(tau + 1, 0)
                                hstage_sin(tau, 1)
                                if tau + 1 < TILES:
                                    hstage_g(tau + 1, 1)
                                    hstage_rrnegi(tau + 1, 1)
                                hstage_proj(tau, 0)
                                hstage_out(tau, 0)
                                hstage_dma(tau, 0)
                                hstage_proj(tau, 1)
                                hstage_out(tau, 1)
                                hstage_dma(tau, 1)
                                continue
                            if _ord == "g_first":
                                hstage_sin(tau, 0)
                                hstage_sin(tau, 1)
                                emit_gsc()
                                if tau + 1 < TILES:
                                    prep_g2(tau + 1)
                                    hstage_g(tau + 1, 0)
                                    hstage_rrnegi(tau + 1, 0)
                                if tau + LA < TILES:
                                    emit_d2exp(
                                        LAST_BLK_OF_TILE[tau + LA] + 1
                                    )
                                if tau + 1 < TILES:
                                    hstage_g(tau + 1, 1)
                                    hstage_rrnegi(tau + 1, 1)
                                hstage_proj(tau, 0)
                                hstage_out(tau, 0)
                                hstage_proj(tau, 1)
                                hstage_out(tau, 1)
                                hstage_dma(tau)
                                continue
                            if _ord == "exp_first":
                                mark = e_emitted[0]
                                if tau + LA < TILES:
                                    emit_d2exp(
                                        LAST_BLK_OF_TILE[tau + LA] + 1
                                    )
                                hstage_sin(tau, 0)
                                hstage_sin(tau, 1)
                                emit_gsc(mark)
                                if tau + 1 < TILES:
                                    prep_g2(tau + 1)
                            elif _ord == "d2_first":
                                if FUSE_SIN:
                                    hstage_sin(tau, None)
                                else:
                                    hstage_sin(tau, 0)
                                    hstage_sin(tau, 1)
                                mark = e_emitted[0]
                                if tau + LA < TILES:
                                    _ex = VARIANT.get("la_extra", 0)
                                    if tau < VARIANT.get("la_early_n", 0):
                                        _ex += VARIANT.get("la_early", 0)
                                    emit_d2exp(min(
                                        LAST_BLK_OF_TILE[tau + LA] + 1
                                        + _ex,
                                        BLOCKS,
                                    ))
                                emit_gsc(mark)
                                if tau + 1 < TILES:
                                    prep_g2(tau + 1)
                            else:
                                hstage_sin(tau, 0)
                                emit_gsc()
                                if tau + 1 < TILES:
                                    prep_g2(tau + 1)
                                hstage_sin(tau, 1)
                                if tau + LA < TILES:
                                    emit_d2exp(
                                        LAST_BLK_OF_TILE[tau + LA] + 1
                                    )
                            if tau + 1 < TILES:
                                if VARIANT.get("negi_late", False):
                                    # both rounds overlap PE's second-half
                                    # group matmuls; negIs then hit with rr
                                    # already computed
                                    hstage_g(tau + 1, 0)
                                    hstage_rr1(tau + 1, 0)
                                    hstage_g(tau + 1, 1)
                                    hstage_rr1(tau + 1, 1)
                                    hstage_negi1(tau + 1, 0)
                                    hstage_negi1(tau + 1, 1)
                                else:
                                    hstage_g(tau + 1, 0)
                                    hstage_rrnegi(tau + 1, 0)
                                    hstage_g(tau + 1, 1)
                                    hstage_rrnegi(tau + 1, 1)
                            if VARIANT.get("dma_half", False):
                                hstage_proj(tau, 0)
                                hstage_out(tau, 0)
                                hstage_dma(tau, 0)
                                hstage_proj(tau, 1)
                                hstage_out(tau, 1)
                                hstage_dma(tau, 1)
                            else:
                                hstage_proj(tau, 0)
                                hstage_out(tau, 0)
                                hstage_proj(tau, 1)
                                hstage_out(tau, 1)
                                hstage_dma(tau)
                    if bench:
                        nc.sync.dma_start(dum_d[:], ob_t[0][0:1, 0:8])

                for rep in range(repeat if not HALF else 0):
                    e_emitted[0] = 0
                    g_done[0] = 0
                    emit_d2exp(LAST_BLK_OF_TILE[min(LA - 1, TILES - 1)] + 1)
                    emit_gsc()
                    prep_g2(0)
                    stage_g(0)
                    if not VARIANT.get("no_rr", False):
                        stage_rrfrac(0)
                    ORDER = VARIANT.get(
                        "iter_order", "sin,gsc,prep,d2,g,frac,proj,out"
                    ).split(",")
                    SKIP = set(VARIANT.get("skip", "").split(","))
                    for tau in range(TILES):
                        for tok in ORDER:
                            if tok in SKIP:
                                continue
                            if tok == "sin":
                                stage_sin(tau)
                            elif tok == "gsc":
                                emit_gsc()
                            elif tok == "prep" and tau + 1 < TILES:
                                prep_g2(tau + 1)
                            elif tok == "d2" and tau + LA < TILES:
                                emit_d2exp(LAST_BLK_OF_TILE[tau + LA] + 1)
                            elif tok == "g" and tau + 1 < TILES:
                                stage_g(tau + 1)
                            elif tok == "frac" and tau + 1 < TILES:
                                if not VARIANT.get("no_rr", False):
                                    stage_rrfrac(tau + 1)
                            elif tok == "proj":
                                stage_proj(tau)
                            elif tok == "out":
                                stage_out(tau)
                if bench:
                    nc.sync.dma_start(dum_d[:], ob_t[0][0:1, 0:8])

            GROUP = VARIANT["group"] if VARIANT["sched"] != "flat" else None
            for rep in range(repeat if GROUP is not None else 0):
              next_blk = 0
              blk_scores.clear(); blk_invrs.clear()
              for g0 in range(0, TILES, GROUP):
                gtiles = range(g0, min(g0 + GROUP, TILES))
                last_blk = max(
                    b for (t, b, d) in CONTRIBS if t in gtiles
                )
                while next_blk <= last_blk:
                    process_block(next_blk, rep)
                    next_blk += 1
                STG = VARIANT["stage"]
                for tau in gtiles:
                    if STG < 2:
                        continue
                    conts = [(b, d) for (t, b, d) in CONTRIBS if t == tau]
                    t_psum = tps_t[0]
                    for ci, (blk, dlt) in enumerate(conts):
                        g2rt = g2rt_t[g2ctr[0] % NG2]
                        g2ctr[0] += 1
                        di = didx[dlt]
                        if VARIANT["gpsimd"]:
                            nc.gpsimd.tensor_scalar(
                                g2rt[:], tG2c[:, di * 128 : (di + 1) * 128],
                                blk_invrs[blk][:], None, ALU.mult,
                            )
                        else:
                            nc.vector.tensor_scalar(
                                g2rt[:], tG2c[:, di * 128 : (di + 1) * 128],
                                blk_invrs[blk][:], None, ALU.mult,
                            )
                        for c0, c1 in CH:
                            nc.tensor.matmul(
                                t_psum[:, c0:c1], g2rt[:],
                                blk_scores[blk][:, c0:c1],
                                start=(ci == 0), stop=False, skip_group_check=True,
                            )
                    if not ACT_BIAS:
                        for c0, c1 in CH:
                            nc.tensor.matmul(
                                t_psum[:, c0:c1], tbp2[:], tones[:, c0:c1],
                                start=False, stop=False, skip_group_check=True,
                            )
                    if STG < 3:
                        continue
                    rr = rr_t[tau % NRR]
                    rr_eng = (nc.gpsimd if VARIANT["rr_eng"] == "pool"
                              else nc.vector)
                    if ACT_BIAS:
                        rr_eng.tensor_scalar(
                            rr[:], t_psum[:, 0:M], tbpc[:, 0:1], MAGIC,
                            ALU.add, ALU.subtract
                        )
                    else:
                        rr_eng.tensor_scalar(
                            rr[:], t_psum[:, 0:M], MAGIC, MAGIC,
                            ALU.add, ALU.subtract
                        )
                    for c0, c1 in CH:
                        nc.tensor.matmul(
                            t_psum[:, c0:c1], tnegI[:], rr[:, c0:c1],
                            start=False, stop=True, skip_group_check=True,
                        )
                    if STG < 4:
                        continue
                    feats = ft_t[tau % NRR]
                    _sf = VARIANT["sin_func"]
                    if _sf == "sin":
                        nc.scalar.activation(
                            feats[:], t_psum[:, 0:M], AF.Sin,
                            scale=float(TWO_PI),
                            bias=tbpc[:, 2:3] if ACT_BIAS else 0.0,
                        )
                    elif _sf == "sin2pi":
                        nc.scalar.activation(
                            feats[:], t_psum[:, 0:M], AF.Arctan,
                            bias=tbpc[:, 1:2] if ACT_BIAS else 0.0,
                        )
                    else:
                        nc.scalar.activation(feats[:], t_psum[:, 0:M], AF.Exp)
                    if STG < 5:
                        continue
                    outp = op_t[0]
                    for c0, c1 in CH:
                        nc.tensor.matmul(
                            outp[:, c0:c1], tW2c[:], feats[:, c0:c1],
                            start=True, stop=True,
                        )
                    out_sb = ob_t[tau % NRR]
                    out_eng = (nc.gpsimd if VARIANT["out_eng"] == "pool"
                               else nc.vector)
                    out_eng.tensor_scalar(
                        out_sb[:, :], outp[:, 0:M], tbo[:], 0.0, ALU.add, ALU.max
                    )
                    if STG < 6 and not (tau == 0 and rep == 0):
                        continue
                    if VARIANT["dma_out"] or (tau == 0 and rep == 0):
                        nc.sync.dma_start(
                            bass.AP(
                                out_d,
                                (NPT * tau * M * H) if VARIANT["dma_out"] else 0,
                                [[M, NPT * H], [1, M]],
                            ),
                            out_sb[0 : NPT * H, :],
                        )
            if bench and VARIANT["sched"] != "flat":
                nc.sync.dma_start(dum_d[:], ob_t[0][0:1, 0:8])

    _split_sync_waits(nc)
    _prog_cache[key] = nc
    return nc


# ---------------------------------------------------------------------------
# host-side input preparation
# ---------------------------------------------------------------------------
def _split3_f16(x):
    x = x.astype(np.float64)
    h = x.astype(np.float16)
    rem = x - h.astype(np.float64)
    m = rem.astype(np.float16)
    rem2 = rem - m.astype(np.float64)
    l = rem2.astype(np.float16)
    return h, m, l


def _tail_fit():
    """LSQ-fold low-freq features onto {1, x, hard sines} over x in [0,1].

    Returns (fit[16, 10], exact_slot[16]) where columns of fit are
    [const, x, s0, c0, s1, c1, s2, c2, s3, c3]."""
    x = np.linspace(0.0, 1.0, 6001)
    Bm = np.stack(
        [np.ones_like(x), x]
        + [fn(FREQS[k] * x) for k in range(4) for fn in (np.sin, np.cos)],
        axis=1,
    )
    fit = np.zeros((16, 10))
    for j in range(16):
        k = j // 2
        fn = np.sin if j % 2 == 0 else np.cos
        if k < 4:
            continue
        tgt = fn(FREQS[k] * x)
        w = np.ones_like(x)
        coef = None
        for _ in range(12):          # Lawson-style reweighting toward minimax
            Wm = Bm * w[:, None]
            coef, *_ = np.linalg.lstsq(Wm, tgt * w, rcond=None)
            err = np.abs(Bm @ coef - tgt)
            w *= (0.2 + err / (err.max() + 1e-18))
            w /= w.mean()
        fit[j] = coef
    return fit


def _prep_core_inputs(queries, crp, vote_w, vote_b, proj_w, proj_b):
    votes = (queries.astype(np.float32) @ vote_w.T.astype(np.float32)
             + vote_b.astype(np.float32)).reshape(B, N, V, 2)
    vx = crp[:, :, 0][:, :, None] + votes[..., 0]
    vy = crp[:, :, 1][:, :, None] + votes[..., 1]
    sq = vx.astype(np.float64) ** 2 + vy.astype(np.float64) ** 2
    X = crp[:, :, 0].astype(np.float64)
    Y = crp[:, :, 1].astype(np.float64)
    u = -2.0 * X
    w_ = -2.0 * Y
    c2 = X * X + Y * Y
    sigma = (crp[:, :, 2] + crp[:, :, 3]).astype(np.float64) / 4.0
    inv2s2 = 1.0 / (2.0 * sigma**2)

    g = SLOT_FREQ / TWO_PI
    bp_slots = (SLOT_FREQ + SLOT_PHASE) / TWO_PI

    # G2c patterns per delta: G2c[q, p] = -g[s] iff delta + q%8 == p//NF
    G2c = np.zeros((128, len(DELTAS) * 128), np.float32)
    for di, dlt in enumerate(DELTAS):
        for q in range(128):
            nloc = dlt + (q % 8)
            if 0 <= nloc < NPT:
                for s in range(NF):
                    G2c[q, di * 128 + nloc * NF + s] = -g[s]
    GSc = (np.arange(128)[:, None] % 8 == np.arange(128)[None, :] % 8).astype(
        np.float32
    )

    # W2 weights: fold tail features into kept slots
    pw = proj_w.astype(np.float64)
    pb = proj_b.astype(np.float64)
    fit = _tail_fit()
    w2 = np.zeros((H, NF))
    bo_h = pb.copy()
    for j in range(16):
        k = j // 2
        if k < 4:
            slot = j            # sin_k -> 2k, cos_k -> 2k+1
            w2[:, slot] += pw[:, j]
        else:
            bo_h += pw[:, j] * fit[j, 0]
            w2[:, 8] += pw[:, j] * fit[j, 1] / EPS_LIN   # x via sin(eps x)/eps
            for sl in range(8):
                w2[:, sl] += pw[:, j] * fit[j, 2 + sl]
    W2c = np.zeros((128, 128), np.float16)
    for nloc in range(NPT):
        p0 = nloc * NF
        o0 = nloc * H
        W2c[p0 : p0 + NF, o0 : o0 + H] = w2.T.astype(np.float16)
    bo = np.zeros((128, 1), np.float32)
    bo[: NPT * H, 0] = np.tile(bo_h, NPT).astype(np.float32)
    bp_arr = np.zeros((2, 128), np.float32)
    bp_arr[0, : NPT * NF] = np.tile(bp_slots, NPT).astype(np.float32)
    bpc = np.zeros((128, 3), np.float32)
    bpc[:, 0] = MAGIC
    bpc[: NPT * NF, 1] = np.tile(bp_slots, NPT).astype(np.float32)
    bpc[:, 2] = bpc[:, 1] * TWO_PI
    negI = np.zeros((128, 128), np.float32)
    np.fill_diagonal(negI, -1.0)
    negI[NPT * NF :, NPT * NF :] = 0.0

    in_maps = []
    for core in range(N_CORES):
        b = core // 2
        nh = core % 2
        n0 = nh * NH

        uh, um, ul = _split3_f16(u[b])
        vh, vm, vl = _split3_f16(w_[b])
        ch, cm, cl = _split3_f16(c2[b])
        ones = np.ones(M, np.float16)
        R = np.stack([uh, um, ul, uh, um, uh,
                      vh, vm, vl, vh, vm, vh,
                      ones, ones, ones,
                      ch, cm, cl]).astype(np.float16)

        n_loc_all = np.arange(NB_PAD)
        n_idx = np.minimum(n0 + n_loc_all, N - 1)
        pad_mask = (n0 + n_loc_all) >= min(n0 + NH, N)
        xb = np.where(pad_mask[:, None], 0.0, vx[b][n_idx])
        yb = np.where(pad_mask[:, None], 0.0, vy[b][n_idx])
        sqb = np.where(pad_mask[:, None], 0.0, sq[b][n_idx])

        def to_cols(a):
            a3 = a.reshape(BLOCKS, 8, V)
            return np.transpose(a3, (0, 2, 1)).reshape(-1)

        xh, xm, xl = _split3_f16(to_cols(xb))
        yh, ym, yl = _split3_f16(to_cols(yb))
        sh, sm, sl = _split3_f16(to_cols(sqb))
        onesL = np.ones(BLOCKS * 128, np.float16)
        L = np.stack([xh, xh, xh, xm, xm, xl,
                      yh, yh, yh, ym, ym, yl,
                      sh, sm, sl,
                      onesL, onesL, onesL]).astype(np.float16)

        nn = n0 + n_loc_all
        vv = np.arange(V)
        i_glob = nn[:, None] * V + vv[None, :]
        sig_idx = i_glob % N
        scv = -inv2s2[b][sig_idx]
        scv = np.where(pad_mask[:, None], -1.0, scv)
        s3 = scv.reshape(BLOCKS, 8, V)
        Sc = np.transpose(s3, (2, 1, 0)).reshape(128, BLOCKS).astype(np.float32)

        in_maps.append({
            "L": L, "R": R, "Sc": Sc, "G2c": G2c, "GSc": GSc, "W2c": W2c,
            "bp": bp_arr, "bpc": bpc, "bo": bo, "negI": negI,
        })
    return in_maps


LAST_RESULTS = None


def kernel(queries, current_ref_points, prev_ref_points, vote_w, vote_b,
           proj_w, proj_b):
    global LAST_RESULTS
    _install_compile_patch()
    queries = np.asarray(queries, np.float32)
    crp = np.asarray(current_ref_points, np.float32)
    in_maps = _prep_core_inputs(
        queries, crp, np.asarray(vote_w, np.float32),
        np.asarray(vote_b, np.float32), np.asarray(proj_w, np.float32),
        np.asarray(proj_b, np.float32),
    )
    nc = _build_program()
    res = run_bass_kernel_spmd(nc, in_maps, list(range(N_CORES)))
    LAST_RESULTS = res
    out = np.empty((B, H, N, M), np.float32)
    for core in range(N_CORES):
        b = core // 2
        nh = core % 2
        o = res.results[core]["out"]          # [NC_PAD, H, M]
        out[b, :, nh * NH : (nh + 1) * NH, :] = np.transpose(
            o[:NH], (1, 0, 2)
        )
    return out

